# revision 18
# baseline (speedup 1.0000x reference)
"""Trainium2 Bass kernel for EvalBspPrime: derivative of degree-16 B-spline
(Bernstein) basis on [0,1].

out[n, m] = 16 * (b_{m-1}(y) - b_m(y)),  b_k(y) = C(15,k) y^k (1-y)^{15-k}

Factorized form (stable, all-multiplicative, no cancellation):
  out_m = sigma_m * (u - m/16) * E_{m-1},  sigma_m = -16*(C(15,m-1)+C(15,m))
  E_j   = u^j v^{14-j}  (degree-14 mixed monomials, v = 1-u)
with edges out_0 = -16 v^15 = (-v/16) * (256 E_0),
           out_16 = 16 u^15 = (u/16) * (256 E_14).

E ladder: G (deg 3) -> F = G*{v^4,u^4} (deg 7) -> E_even = Square(F) on the
scalar engine (|sigma| folded in via the Square's input scale), E_odd =
F_i*F_{i+1} on the vector engine. All output writes are contiguous
(k-major); the host interleaves to [N, 17] at the end.

Sharding: elementwise over N -> N/8 per core (data parallel), no
communication. GPSIMD is left idle on purpose: it shares an SBUF port with
the DVE and concurrent use measured DVE 2-tensor ops at ~2x cost.
"""

import math
import os
import sys

import numpy as np

for _p in ("/opt/trn_rl_repo",):
    if os.path.isdir(_p) and _p not in sys.path:
        sys.path.insert(0, _p)

import concourse.bacc as bacc
import concourse.bass as bass
import concourse.mybir as mybir
from concourse import tile
from concourse.bass_utils import run_bass_kernel_spmd

P = 128          # SBUF partitions
COLS = 3907      # free-dim columns per core
CORE_PTS = P * COLS          # 500096 points per core
NCORES = 8
PAD_N = CORE_PTS * NCORES    # 4000768
K = 17           # ORDER + 1 outputs per point
FMAX = 768       # free-dim tile size

_BINOM = [1.0, 15.0, 105.0, 455.0, 1365.0, 3003.0, 5005.0, 6435.0,
          6435.0, 5005.0, 3003.0, 1365.0, 455.0, 105.0, 15.0, 1.0]
# |sigma_m| = 16*(C(15,m-1)+C(15,m)) for m=1..15
_ASIG = [16.0 * (_BINOM[m - 1] + _BINOM[m]) for m in range(1, 16)]


def _chunks():
    out = []
    c0 = 0
    while c0 < COLS:
        f = min(FMAX, COLS - c0)
        out.append((c0, f))
        c0 += f
    return out


def _build_nc():
    nc = bacc.Bacc("TRN2", target_bir_lowering=False, debug=False,
                   num_devices=NCORES)
    f32 = mybir.dt.float32
    AF = mybir.ActivationFunctionType
    OP = mybir.AluOpType
    X = nc.declare_dram_parameter("x", [P, COLS], f32, isOutput=False)
    # k-major output: [P, 17 * COLS]; host interleaves to [N, 17]
    OUT = nc.declare_dram_parameter("out", [P, K * COLS], f32, isOutput=True)
    OUT_K = OUT.ap().rearrange("p (k c) -> p k c", k=K)

    with tile.TileContext(nc) as tc:
        with (
            tc.tile_pool(name="io", bufs=2) as io_pool,
            tc.tile_pool(name="pw", bufs=1) as pw,
            tc.tile_pool(name="wb", bufs=4) as wb,
            tc.tile_pool(name="ob", bufs=2) as ob,
        ):
            for (c0, F) in _chunks():
                t = {}
                u = io_pool.tile([P, F], f32, tag="u")
                nc.sync.dma_start(u[:], X[:, c0:c0 + F])
                t["u"] = u

                def act(dst, src, func, scale=1.0, bias=0.0):
                    tl = pw.tile([P, F], f32, tag=dst)
                    nc.scalar.activation(tl[:], src[:], func,
                                         bias=float(bias), scale=float(scale))
                    t[dst] = tl

                def v_mul(dst, a, b):
                    tl = pw.tile([P, F], f32, tag=dst)
                    nc.vector.tensor_tensor(tl[:], t[a][:], t[b][:], OP.mult)
                    t[dst] = tl

                # --- scalar-engine unary ladder ---
                act("v", u, AF.Copy, -1.0, 1.0)        # v = 1-u
                act("nu", u, AF.Copy, -1.0, 0.0)       # -u
                act("vo16", u, AF.Copy, 1.0 / 16.0, -1.0 / 16.0)  # (u-1)/16 = -v/16
                act("uo16", u, AF.Copy, 1.0 / 16.0, 0.0)          # u/16
                act("u2", u, AF.Square)
                act("v2", u, AF.Square, -1.0, 1.0)     # (1-u)^2
                act("u4", t["u2"], AF.Square)
                act("v4", t["v2"], AF.Square)

                # --- degree-3 monomials (DVE) ---
                v_mul("g0", "v", "v2")    # v^3
                v_mul("g1", "u", "v2")    # u v^2
                v_mul("g2", "v", "u2")    # u^2 v
                v_mul("g3", "u", "u2")    # u^3

                # --- degree-7 monomials F_i = u^i v^(7-i) (DVE) ---
                for i in range(4):
                    v_mul(f"f{i}", f"g{i}", "v4")
                for i in range(4):
                    v_mul(f"f{i + 4}", f"g{i}", "u4")

                # --- scaled degree-14 monomials Etil_j = |sigma_{j+1}| E_j ---
                # even j: Square(sqrt|sigma| * F_{j/2}) on ACT
                for j in range(0, 15, 2):
                    s = math.sqrt(_ASIG[j])
                    act(f"e{j}", t[f"f{j // 2}"], AF.Square, s, 0.0)

                out_t = ob.tile([P, F * K], f32, tag="out")

                def out_slice(m):
                    return out_t[:, m * F:(m + 1) * F]

                # odd j: (F_a * |sigma|) * F_{a+1} on DVE (STT), then finals.
                # Emit odd-E and finals interleaved so tiles free quickly.
                for j in range(1, 14, 2):
                    a = (j - 1) // 2
                    tl = wb.tile([P, F], f32, tag="w")
                    nc.vector.scalar_tensor_tensor(
                        tl[:], t[f"f{a}"][:], _ASIG[j], t[f"f{a + 1}"][:],
                        OP.mult, OP.mult)
                    t[f"e{j}"] = tl

                # finals: out_m = (-u + m/16) * Etil_{m-1}, m = 1..15
                for m in range(1, 16):
                    nc.vector.scalar_tensor_tensor(
                        out_slice(m), t["nu"][:], m / 16.0, t[f"e{m - 1}"][:],
                        OP.add, OP.mult)
                # out_0 = (-v/16) * Etil_0;  out_16 = (u/16) * Etil_14
                nc.vector.tensor_tensor(
                    out_slice(0), t["vo16"][:], t["e0"][:], OP.mult)
                nc.vector.tensor_tensor(
                    out_slice(16), t["uo16"][:], t["e14"][:], OP.mult)

                nc.sync.dma_start(OUT_K[:, :, c0:c0 + F],
                                  out_t[:].rearrange("p (k c) -> p k c", k=K))
    nc.finalize()
    return nc


_CACHE = {}


def _run(x, trace=False, trace_kwargs=None):
    x = np.ascontiguousarray(np.asarray(x, dtype=np.float32))
    n = x.shape[0]
    xf = x.reshape(-1)
    pad = PAD_N - n
    if pad:
        xf = np.concatenate([xf, np.full(pad, 0.5, np.float32)])
    shards = xf.reshape(NCORES, P, COLS)
    if "nc" not in _CACHE:
        _CACHE["nc"] = _build_nc()
    nc = _CACHE["nc"]
    in_maps = [{"x": np.ascontiguousarray(shards[i])} for i in range(NCORES)]
    kw = {}
    if trace:
        kw["trace"] = True
        if trace_kwargs:
            kw.update(trace_kwargs)
    res = run_bass_kernel_spmd(nc, in_maps, list(range(NCORES)), **kw)
    outs = res.results
    full = np.empty((PAD_N, K), dtype=np.float32)
    for i in range(NCORES):
        o = np.asarray(outs[i]["out"]).reshape(P, K, COLS)
        # [P, K, COLS] -> [P, COLS, K] -> [CORE_PTS, K]
        full[i * CORE_PTS:(i + 1) * CORE_PTS] = (
            o.transpose(0, 2, 1).reshape(CORE_PTS, K))
    return full[:n], res


def kernel(x):
    out, _ = _run(x)
    return out


# revision 19
# speedup vs baseline: 1.0024x; 1.0024x over previous
"""Trainium2 Bass kernel for EvalBspPrime: derivative of degree-16 B-spline
(Bernstein) basis on [0,1].

out[n, m] = 16 * (b_{m-1}(y) - b_m(y)),  b_k(y) = C(15,k) y^k (1-y)^{15-k}

Factorized form (stable, all-multiplicative, no cancellation):
  out_m = sigma_m * (u - m/16) * E_{m-1},  sigma_m = -16*(C(15,m-1)+C(15,m))
  E_j   = u^j v^{14-j}  (degree-14 mixed monomials, v = 1-u)
with edges out_0 = -16 v^15 = (-v/16) * (256 E_0),
           out_16 = 16 u^15 = (u/16) * (256 E_14).

E ladder: G (deg 3) -> F = G*{v^4,u^4} (deg 7) -> E_even = Square(F) on the
scalar engine (|sigma| folded in via the Square's input scale), E_odd =
F_i*F_{i+1} on the vector engine. All output writes are contiguous
(k-major); the host interleaves to [N, 17] at the end.

Sharding: elementwise over N -> N/8 per core (data parallel), no
communication. GPSIMD is left idle on purpose: it shares an SBUF port with
the DVE and concurrent use measured DVE 2-tensor ops at ~2x cost.
"""

import math
import os
import sys

import numpy as np

for _p in ("/opt/trn_rl_repo",):
    if os.path.isdir(_p) and _p not in sys.path:
        sys.path.insert(0, _p)

import concourse.bacc as bacc
import concourse.bass as bass
import concourse.mybir as mybir
from concourse import tile
from concourse.bass_utils import run_bass_kernel_spmd

P = 128          # SBUF partitions
COLS = 3907      # free-dim columns per core
CORE_PTS = P * COLS          # 500096 points per core
NCORES = 8
PAD_N = CORE_PTS * NCORES    # 4000768
K = 17           # ORDER + 1 outputs per point
FMAX = 768       # free-dim tile size

_BINOM = [1.0, 15.0, 105.0, 455.0, 1365.0, 3003.0, 5005.0, 6435.0,
          6435.0, 5005.0, 3003.0, 1365.0, 455.0, 105.0, 15.0, 1.0]
# |sigma_m| = 16*(C(15,m-1)+C(15,m)) for m=1..15
_ASIG = [16.0 * (_BINOM[m - 1] + _BINOM[m]) for m in range(1, 16)]


def _chunks():
    # Small first chunk (short pipeline ramp), big middles (amortize per-op
    # overhead), descending tail (the last DMAs overlap the shrinking
    # compute; the final un-overlapped DMA is tiny).
    if COLS == 3907 and FMAX == 768:
        sizes = [384, 672, 672, 672, 672, 512, 256, 67]
    else:
        sizes = []
        c = COLS
        while c > 0:
            f = min(FMAX, c)
            sizes.append(f)
            c -= f
    out = []
    c0 = 0
    for f in sizes:
        out.append((c0, f))
        c0 += f
    assert c0 == COLS
    return out


def _build_nc():
    nc = bacc.Bacc("TRN2", target_bir_lowering=False, debug=False,
                   num_devices=NCORES)
    f32 = mybir.dt.float32
    AF = mybir.ActivationFunctionType
    OP = mybir.AluOpType
    X = nc.declare_dram_parameter("x", [P, COLS], f32, isOutput=False)
    # k-major output: [P, 17 * COLS]; host interleaves to [N, 17]
    OUT = nc.declare_dram_parameter("out", [P, K * COLS], f32, isOutput=True)
    OUT_K = OUT.ap().rearrange("p (k c) -> p k c", k=K)

    with tile.TileContext(nc) as tc:
        with (
            tc.tile_pool(name="io", bufs=2) as io_pool,
            tc.tile_pool(name="pw", bufs=1) as pw,
            tc.tile_pool(name="wb", bufs=4) as wb,
            tc.tile_pool(name="ob", bufs=2) as ob,
        ):
            for (c0, F) in _chunks():
                t = {}
                u = io_pool.tile([P, F], f32, tag="u")
                nc.sync.dma_start(u[:], X[:, c0:c0 + F])
                t["u"] = u

                def act(dst, src, func, scale=1.0, bias=0.0):
                    tl = pw.tile([P, F], f32, tag=dst)
                    nc.scalar.activation(tl[:], src[:], func,
                                         bias=float(bias), scale=float(scale))
                    t[dst] = tl

                def v_mul(dst, a, b):
                    tl = pw.tile([P, F], f32, tag=dst)
                    nc.vector.tensor_tensor(tl[:], t[a][:], t[b][:], OP.mult)
                    t[dst] = tl

                # --- scalar-engine unary ladder ---
                act("v", u, AF.Copy, -1.0, 1.0)        # v = 1-u
                act("nu", u, AF.Copy, -1.0, 0.0)       # -u
                act("vo16", u, AF.Copy, 1.0 / 16.0, -1.0 / 16.0)  # (u-1)/16 = -v/16
                act("uo16", u, AF.Copy, 1.0 / 16.0, 0.0)          # u/16
                act("u2", u, AF.Square)
                act("v2", u, AF.Square, -1.0, 1.0)     # (1-u)^2
                act("u4", t["u2"], AF.Square)
                act("v4", t["v2"], AF.Square)

                # --- degree-3 monomials (DVE) ---
                v_mul("g0", "v", "v2")    # v^3
                v_mul("g1", "u", "v2")    # u v^2
                v_mul("g2", "v", "u2")    # u^2 v
                v_mul("g3", "u", "u2")    # u^3

                # --- degree-7 monomials F_i = u^i v^(7-i) (DVE) ---
                for i in range(4):
                    v_mul(f"f{i}", f"g{i}", "v4")
                for i in range(4):
                    v_mul(f"f{i + 4}", f"g{i}", "u4")

                # --- scaled degree-14 monomials Etil_j = |sigma_{j+1}| E_j ---
                # even j: Square(sqrt|sigma| * F_{j/2}) on ACT
                for j in range(0, 15, 2):
                    s = math.sqrt(_ASIG[j])
                    act(f"e{j}", t[f"f{j // 2}"], AF.Square, s, 0.0)

                out_t = ob.tile([P, F * K], f32, tag="out")

                def out_slice(m):
                    return out_t[:, m * F:(m + 1) * F]

                # odd j: (F_a * |sigma|) * F_{a+1} on DVE (STT), then finals.
                # Emit odd-E and finals interleaved so tiles free quickly.
                for j in range(1, 14, 2):
                    a = (j - 1) // 2
                    tl = wb.tile([P, F], f32, tag="w")
                    nc.vector.scalar_tensor_tensor(
                        tl[:], t[f"f{a}"][:], _ASIG[j], t[f"f{a + 1}"][:],
                        OP.mult, OP.mult)
                    t[f"e{j}"] = tl

                # finals: out_m = (-u + m/16) * Etil_{m-1}, m = 1..15
                for m in range(1, 16):
                    nc.vector.scalar_tensor_tensor(
                        out_slice(m), t["nu"][:], m / 16.0, t[f"e{m - 1}"][:],
                        OP.add, OP.mult)
                # out_0 = (-v/16) * Etil_0;  out_16 = (u/16) * Etil_14
                nc.vector.tensor_tensor(
                    out_slice(0), t["vo16"][:], t["e0"][:], OP.mult)
                nc.vector.tensor_tensor(
                    out_slice(16), t["uo16"][:], t["e14"][:], OP.mult)

                nc.sync.dma_start(OUT_K[:, :, c0:c0 + F],
                                  out_t[:].rearrange("p (k c) -> p k c", k=K))
    nc.finalize()
    return nc


_CACHE = {}


def _run(x, trace=False, trace_kwargs=None):
    x = np.ascontiguousarray(np.asarray(x, dtype=np.float32))
    n = x.shape[0]
    xf = x.reshape(-1)
    pad = PAD_N - n
    if pad:
        xf = np.concatenate([xf, np.full(pad, 0.5, np.float32)])
    shards = xf.reshape(NCORES, P, COLS)
    if "nc" not in _CACHE:
        _CACHE["nc"] = _build_nc()
    nc = _CACHE["nc"]
    in_maps = [{"x": np.ascontiguousarray(shards[i])} for i in range(NCORES)]
    kw = {}
    if trace:
        kw["trace"] = True
        if trace_kwargs:
            kw.update(trace_kwargs)
    res = run_bass_kernel_spmd(nc, in_maps, list(range(NCORES)), **kw)
    outs = res.results
    full = np.empty((PAD_N, K), dtype=np.float32)
    for i in range(NCORES):
        o = np.asarray(outs[i]["out"]).reshape(P, K, COLS)
        # [P, K, COLS] -> [P, COLS, K] -> [CORE_PTS, K]
        full[i * CORE_PTS:(i + 1) * CORE_PTS] = (
            o.transpose(0, 2, 1).reshape(CORE_PTS, K))
    return full[:n], res


def kernel(x):
    out, _ = _run(x)
    return out


# revision 22
# speedup vs baseline: 1.0268x; 1.0243x over previous
"""Trainium2 Bass kernel for EvalBspPrime: derivative of degree-16 B-spline
(Bernstein) basis on [0,1].

out[n, m] = 16 * (b_{m-1}(y) - b_m(y)),  b_k(y) = C(15,k) y^k (1-y)^{15-k}

Factorized form (stable, all-multiplicative, no cancellation):
  out_m = sigma_m * (u - m/16) * E_{m-1},  sigma_m = -16*(C(15,m-1)+C(15,m))
  E_j   = u^j v^{14-j}  (degree-14 mixed monomials, v = 1-u)
with edges out_0 = -16 v^15 = (-v/16) * (256 E_0),
           out_16 = 16 u^15 = (u/16) * (256 E_14).

E ladder: G (deg 3) -> F = G*{v^4,u^4} (deg 7) -> E_even = Square(F) on the
scalar engine (|sigma| folded in via the Square's input scale), E_odd =
F_i*F_{i+1} on the vector engine. All output writes are contiguous
(k-major); the host interleaves to [N, 17] at the end.

Sharding: elementwise over N -> N/8 per core (data parallel), no
communication. GPSIMD is left idle on purpose: it shares an SBUF port with
the DVE and concurrent use measured DVE 2-tensor ops at ~2x cost.
"""

import math
import os
import sys

import numpy as np

for _p in ("/opt/trn_rl_repo",):
    if os.path.isdir(_p) and _p not in sys.path:
        sys.path.insert(0, _p)

import concourse.bacc as bacc
import concourse.bass as bass
import concourse.mybir as mybir
from concourse import tile
from concourse.bass_utils import run_bass_kernel_spmd

P = 128          # SBUF partitions
COLS = 3907      # free-dim columns per core
CORE_PTS = P * COLS          # 500096 points per core
NCORES = 8
PAD_N = CORE_PTS * NCORES    # 4000768
K = 17           # ORDER + 1 outputs per point
FMAX = 512       # free-dim tile size

_BINOM = [1.0, 15.0, 105.0, 455.0, 1365.0, 3003.0, 5005.0, 6435.0,
          6435.0, 5005.0, 3003.0, 1365.0, 455.0, 105.0, 15.0, 1.0]
# |sigma_m| = 16*(C(15,m-1)+C(15,m)) for m=1..15
_ASIG = [16.0 * (_BINOM[m - 1] + _BINOM[m]) for m in range(1, 16)]


def _chunks():
    sizes = []
    c = COLS
    while c > 0:
        f = min(FMAX, c)
        sizes.append(f)
        c -= f
    out = []
    c0 = 0
    for f in sizes:
        out.append((c0, f))
        c0 += f
    assert c0 == COLS
    return out


def _build_nc():
    nc = bacc.Bacc("TRN2", target_bir_lowering=False, debug=False,
                   num_devices=NCORES)
    f32 = mybir.dt.float32
    AF = mybir.ActivationFunctionType
    OP = mybir.AluOpType
    X = nc.declare_dram_parameter("x", [P, COLS], f32, isOutput=False)
    # k-major output: [P, 17 * COLS]; host interleaves to [N, 17]
    OUT = nc.declare_dram_parameter("out", [P, K * COLS], f32, isOutput=True)
    OUT_K = OUT.ap().rearrange("p (k c) -> p k c", k=K)

    with tile.TileContext(nc) as tc:
        with (
            tc.tile_pool(name="io", bufs=2) as io_pool,
            tc.tile_pool(name="pw", bufs=2) as pw,
            tc.tile_pool(name="wb", bufs=6) as wb,
            tc.tile_pool(name="ob", bufs=2) as ob,
        ):
            for (c0, F) in _chunks():
                t = {}
                u = io_pool.tile([P, F], f32, tag="u")
                nc.sync.dma_start(u[:], X[:, c0:c0 + F])
                t["u"] = u

                def act(dst, src, func, scale=1.0, bias=0.0):
                    tl = pw.tile([P, F], f32, tag=dst)
                    nc.scalar.activation(tl[:], src[:], func,
                                         bias=float(bias), scale=float(scale))
                    t[dst] = tl

                def v_mul(dst, a, b):
                    tl = pw.tile([P, F], f32, tag=dst)
                    nc.vector.tensor_tensor(tl[:], t[a][:], t[b][:], OP.mult)
                    t[dst] = tl

                # --- scalar-engine unary ladder ---
                act("v", u, AF.Copy, -1.0, 1.0)        # v = 1-u
                act("nu", u, AF.Copy, -1.0, 0.0)       # -u
                act("vo16", u, AF.Copy, 1.0 / 16.0, -1.0 / 16.0)  # (u-1)/16 = -v/16
                act("uo16", u, AF.Copy, 1.0 / 16.0, 0.0)          # u/16
                act("u2", u, AF.Square)
                act("v2", u, AF.Square, -1.0, 1.0)     # (1-u)^2
                act("u4", t["u2"], AF.Square)
                act("v4", t["v2"], AF.Square)

                # --- degree-3 monomials (DVE) ---
                v_mul("g0", "v", "v2")    # v^3
                v_mul("g1", "u", "v2")    # u v^2
                v_mul("g2", "v", "u2")    # u^2 v
                v_mul("g3", "u", "u2")    # u^3

                # --- degree-7 monomials F_i = u^i v^(7-i) (DVE) ---
                for i in range(4):
                    v_mul(f"f{i}", f"g{i}", "v4")
                for i in range(4):
                    v_mul(f"f{i + 4}", f"g{i}", "u4")

                # --- scaled degree-14 monomials Etil_j = |sigma_{j+1}| E_j ---
                # even j: Square(sqrt|sigma| * F_{j/2}) on ACT
                for j in range(0, 15, 2):
                    s = math.sqrt(_ASIG[j])
                    act(f"e{j}", t[f"f{j // 2}"], AF.Square, s, 0.0)

                out_t = ob.tile([P, F * K], f32, tag="out")

                def out_slice(m):
                    return out_t[:, m * F:(m + 1) * F]

                # odd j: (F_a * |sigma|) * F_{a+1} on DVE (STT), then finals.
                # Emit odd-E and finals interleaved so tiles free quickly.
                for j in range(1, 14, 2):
                    a = (j - 1) // 2
                    tl = wb.tile([P, F], f32, tag="w")
                    nc.vector.scalar_tensor_tensor(
                        tl[:], t[f"f{a}"][:], _ASIG[j], t[f"f{a + 1}"][:],
                        OP.mult, OP.mult)
                    t[f"e{j}"] = tl

                # finals: out_m = (-u + m/16) * Etil_{m-1}, m = 1..15
                for m in range(1, 16):
                    nc.vector.scalar_tensor_tensor(
                        out_slice(m), t["nu"][:], m / 16.0, t[f"e{m - 1}"][:],
                        OP.add, OP.mult)
                # out_0 = (-v/16) * Etil_0;  out_16 = (u/16) * Etil_14
                nc.vector.tensor_tensor(
                    out_slice(0), t["vo16"][:], t["e0"][:], OP.mult)
                nc.vector.tensor_tensor(
                    out_slice(16), t["uo16"][:], t["e14"][:], OP.mult)

                nc.sync.dma_start(OUT_K[:, :, c0:c0 + F],
                                  out_t[:].rearrange("p (k c) -> p k c", k=K))
    nc.finalize()
    return nc


_CACHE = {}


def _run(x, trace=False, trace_kwargs=None):
    x = np.ascontiguousarray(np.asarray(x, dtype=np.float32))
    n = x.shape[0]
    xf = x.reshape(-1)
    pad = PAD_N - n
    if pad:
        xf = np.concatenate([xf, np.full(pad, 0.5, np.float32)])
    shards = xf.reshape(NCORES, P, COLS)
    if "nc" not in _CACHE:
        _CACHE["nc"] = _build_nc()
    nc = _CACHE["nc"]
    in_maps = [{"x": np.ascontiguousarray(shards[i])} for i in range(NCORES)]
    kw = {}
    if trace:
        kw["trace"] = True
        if trace_kwargs:
            kw.update(trace_kwargs)
    res = run_bass_kernel_spmd(nc, in_maps, list(range(NCORES)), **kw)
    outs = res.results
    full = np.empty((PAD_N, K), dtype=np.float32)
    for i in range(NCORES):
        o = np.asarray(outs[i]["out"]).reshape(P, K, COLS)
        # [P, K, COLS] -> [P, COLS, K] -> [CORE_PTS, K]
        full[i * CORE_PTS:(i + 1) * CORE_PTS] = (
            o.transpose(0, 2, 1).reshape(CORE_PTS, K))
    return full[:n], res


def kernel(x):
    out, _ = _run(x)
    return out
